# revision 23
# baseline (speedup 1.0000x reference)
"""LPCNet sampling kernel for Trainium2 — nn_LPCNet_91061896609827.

kernel(**inputs) takes the FULL unsharded inputs and returns the FULL
[B, F*T, K] float32 logits, running the T=160-step sampling recurrence on 8
NeuronCores (data-parallel over the R = B*F = 1024 row axis, 128 rows/core,
small weights replicated — per the sharding hint).

Device algorithm (feature-major: features on SBUF partitions, rows on the
free axis, so every matmul uses the static weights as the stationary operand
and needs no activation transposes):

- GRU matmuls with fp16 operands (fp32 PSUM accumulation). The three
  raw-scale feedback features (p, s_last, e_prev; integer-valued up to 510,
  exact in fp16) hit fp16 weight rounding hard, so their Wxa rows are applied
  twice — an fp16-hi and an fp16-lo stationary matrix against the same
  feature tile (features sit at partitions 0/32/64 with the bias ones-row at
  96, because SBUF engine APs must start at a 32-aligned partition).
- sigmoid(x) = 0.5 + 0.5*tanh(x/2), so only {tanh, exp} are needed on the
  scalar engine — one activation table set, zero table switches.
- GRU state is stored doubled (A = 2*ha) with the 0.5 folded into the
  pre-scaled weights: the update is A' = (.5A + n) + t_z*(.5A - n).
- mu-law quantization is computed exactly with 255 precomputed fp32
  thresholds (bisected on the host against the reference formula):
  p = #{j : lpc.prev_s >= thr_j} via PE broadcast + DVE compare + PE count.
- gumbel-softmax: probs ∝ exp(logits)/log(u), so soft = S1/S0 with
  S0 = sum E*w', S1 = sum E*w'*idx, w' = 1/log(u) (host-precomputed,
  uploaded pre-transposed per core). e = round(soft).
- the 16-deep LPC sample ring lives in PSUM (no partition-base limits) and
  is dotted against host-rotated lpc variants, so nothing ever shifts.

Validated against the jax reference by a bit-faithful numpy simulation of
this exact algorithm: rel-err ~1e-3 to 1.8e-3 (gate is 2e-2); decision flips
are damped by the GRU dynamics (measured).

Self-contained: hardcodes all shapes; reads nothing from /root/problem.
"""
import os
import numpy as np

import concourse.bass as bass
import concourse.bacc as bacc
import concourse.tile as tile
from concourse import mybir
from concourse.bass_utils import run_bass_kernel_spmd

B, F, M, NF = 32, 32, 16, 20
T, K = 160, 256
R = B * F
COND, HA, HB = 128, 384, 16
MU = 255.0
N_CORES = 8
RC = R // N_CORES        # 128 rows per core
f32 = np.float32
f16 = np.float16
DT32 = mybir.dt.float32
DT16 = mybir.dt.float16
DTI32 = mybir.dt.int32
TANH = mybir.ActivationFunctionType.Tanh
EXP = mybir.ActivationFunctionType.Exp
ALU = mybir.AluOpType

T_STEPS = int(os.environ.get("LPC_T_STEPS", T))
# round(soft): "rne" = direct fp32->int32 cast — hardware-verified to be
# round-to-nearest-even (= jnp.round). CoreSim truncates instead; use
# LPC_ROUND_MODE=halfup only when running under the simulator.
ROUND_MODE = os.environ.get("LPC_ROUND_MODE", "rne")

last_exec_time_ns = None   # set by kernel() when tracing is enabled


# ----------------------------------------------------------------- host math

def _mu_law_quantize_ref(x):
    xc = np.clip(x, f32(-1.0), f32(1.0)).astype(f32)
    ln_mu1 = np.log(f32(1.0 + MU))
    y = (np.sign(xc) * np.log1p(f32(MU) * np.abs(xc)) / ln_mu1).astype(f32)
    return np.clip(np.floor((y + f32(1.0)) * f32(0.5) * f32(MU + 1.0)),
                   f32(0.0), f32(MU)).astype(f32)


def _compute_thresholds():
    """thr[j], j=0..254: smallest fp32 v with mu_law(v) >= j+1 (monotone)."""
    lo = np.full(255, -2.0, f32)
    hi = np.full(255, 2.0, f32)
    target = np.arange(1, 256, dtype=f32)
    for _ in range(80):
        mid = ((lo.astype(np.float64) + hi.astype(np.float64)) / 2).astype(f32)
        same = (mid == lo) | (mid == hi)
        lev = _mu_law_quantize_ref(mid)
        take_hi = lev >= target
        hi = np.where(take_hi, mid, hi)
        lo = np.where(~take_hi, mid, lo)
        if same.all():
            break
    for _ in range(4):
        cand = np.nextafter(hi, -np.inf, dtype=f32)
        lev = _mu_law_quantize_ref(cand)
        hi = np.where(lev >= target, cand, hi)
    return hi


def _f16x2(w):
    hi = w.astype(f16)
    lo = (w.astype(f32) - hi.astype(f32)).astype(f16)
    return hi, lo


# ------------------------------------------------------------- device kernel

def _ts(i, n=128):
    return slice(i * n, (i + 1) * n)


def _build_nc(t_steps):
    nc = bacc.Bacc()

    # --- per-core dram parameters (inputs)
    d_wT = nc.declare_dram_parameter("wT", [t_steps, RC, 2 * RC], DT32, isOutput=False)
    d_cond = nc.declare_dram_parameter("cond16", [COND, RC], DT16, isOutput=False)
    d_wha = nc.declare_dram_parameter("whaT", [RC, 3 * 3 * HA], DT16, isOutput=False)
    d_wxac = nc.declare_dram_parameter("wxacT", [RC, 3 * HA], DT16, isOutput=False)
    d_dw1 = nc.declare_dram_parameter("dynW1", [97, 3 * HA], DT16, isOutput=False)
    d_dw2 = nc.declare_dram_parameter("dynW2", [97, 3 * HA], DT16, isOutput=False)
    d_wxb = nc.declare_dram_parameter("wxbT", [RC, 3 * 3 * HB], DT16, isOutput=False)
    d_bb2 = nc.declare_dram_parameter("bb2W", [2, 3 * HB], DT16, isOutput=False)
    d_whb = nc.declare_dram_parameter("whbT", [HB, 3 * HB], DT16, isOutput=False)
    d_w12 = nc.declare_dram_parameter("w12T", [HB + 2, 2 * K], DT16, isOutput=False)
    d_lpcr = nc.declare_dram_parameter("lpcfm", [M, RC], DT32, isOutput=False)
    d_shift = nc.declare_dram_parameter("shiftm", [M, 2 * M], DT32, isOutput=False)
    d_thrp = nc.declare_dram_parameter("thrp", [RC, 2], DT32, isOutput=False)
    d_sidx = nc.declare_dram_parameter("sidx", [RC, 4], DT32, isOutput=False)
    d_gg = nc.declare_dram_parameter("ggcol", [RC, 4], DT32, isOutput=False)
    d_out = nc.declare_dram_parameter("out_lg", [t_steps, RC, 2 * RC], DT32, isOutput=True)
    debug = bool(int(os.environ.get("LPC_DEBUG", "0")))
    d_dbg = None
    if debug:
        d_dbg = nc.declare_dram_parameter("dbg", [t_steps, 4, RC], DT32, isOutput=True)

    with tile.TileContext(nc) as tc:
        with (
            tc.tile_pool(name="singles", bufs=1) as singles,
            tc.tile_pool(name="wpool", bufs=4) as wpool,
            tc.tile_pool(name="lgpool", bufs=4) as lgpool,
            tc.tile_pool(name="work", bufs=2) as work,
            tc.tile_pool(name="psA", bufs=1, space="PSUM") as psA,
            tc.tile_pool(name="psB", bufs=1, space="PSUM") as psB,
            tc.tile_pool(name="psC", bufs=1, space="PSUM") as psC,
            tc.tile_pool(name="psD", bufs=1, space="PSUM") as psD,
            tc.tile_pool(name="psE", bufs=1, space="PSUM") as psE,
            tc.tile_pool(name="psF", bufs=1, space="PSUM") as psF,
            tc.tile_pool(name="psR", bufs=1, space="PSUM") as psR,
        ):
            # --- load constants / weights into SBUF
            sb_cond = singles.tile([COND, RC], DT16)
            nc.sync.dma_start(out=sb_cond, in_=d_cond[:, :])
            sb_wha = singles.tile([RC, 3, 3 * HA], DT16)
            nc.sync.dma_start(out=sb_wha, in_=d_wha[:, :].rearrange("p (c f) -> p c f", c=3))
            sb_wxac = singles.tile([RC, 3 * HA], DT16)
            nc.sync.dma_start(out=sb_wxac, in_=d_wxac[:, :])
            sb_dw1 = singles.tile([97, 3 * HA], DT16)
            nc.sync.dma_start(out=sb_dw1, in_=d_dw1[:, :])
            sb_dw2 = singles.tile([97, 3 * HA], DT16)
            nc.sync.dma_start(out=sb_dw2, in_=d_dw2[:, :])
            sb_wxb = singles.tile([RC, 3, 3 * HB], DT16)
            nc.sync.dma_start(out=sb_wxb, in_=d_wxb[:, :].rearrange("p (c f) -> p c f", c=3))
            sb_bb2 = singles.tile([2, 3 * HB], DT16)
            nc.sync.dma_start(out=sb_bb2, in_=d_bb2[:, :])
            sb_whb = singles.tile([HB, 3 * HB], DT16)
            nc.sync.dma_start(out=sb_whb, in_=d_whb[:, :])
            sb_w12 = singles.tile([HB + 2, 2 * K], DT16)
            nc.sync.dma_start(out=sb_w12, in_=d_w12[:, :])
            sb_lpcr = singles.tile([M, RC], DT32)
            nc.sync.dma_start(out=sb_lpcr, in_=d_lpcr[:, :])
            # shiftm cols 0..15: shift matrix; cols 16..31 row 0: unit(15)
            sb_shift = singles.tile([M, 2 * M], DT32)
            nc.sync.dma_start(out=sb_shift, in_=d_shift[:, :])
            sb_thrp = singles.tile([RC, 2], DT32)
            nc.sync.dma_start(out=sb_thrp, in_=d_thrp[:, :])
            sb_sidx = singles.tile([RC, 4], DT32)
            nc.sync.dma_start(out=sb_sidx, in_=d_sidx[:, :])
            sb_gg = singles.tile([RC, 4], DT32)
            nc.sync.dma_start(out=sb_gg, in_=d_gg[:, :])

            sb_ones16 = singles.tile([M, RC], DT32)
            nc.vector.memset(sb_ones16, 1.0)
            sb_ones128 = singles.tile([RC, 1], DT16)
            nc.vector.memset(sb_ones128, 1.0)
            sb_ones2 = singles.tile([2, RC], DT16)
            nc.vector.memset(sb_ones2, 1.0)

            # --- state tiles
            st_A16 = singles.tile([RC, HA], DT16)        # doubled GRU-A state
            nc.vector.memset(st_A16, 0.0)
            st_B32 = singles.tile([HB, RC], DT32)        # doubled GRU-B state
            nc.vector.memset(st_B32, 0.0)
            st_hbE = singles.tile([HB + 2, RC], DT16)    # [B16; 1; 1]
            nc.vector.memset(st_hbE, 1.0)
            nc.vector.memset(st_hbE[:HB, :], 0.0)
            # dyn features: p@0, s@32, e@64, ones@96 (32-aligned SBUF bases)
            st_dynX = singles.tile([97, RC], DT16)
            nc.vector.memset(st_dynX, 0.0)
            nc.vector.memset(st_dynX[0:1, :], 128.0)     # p(0) = mulaw(0) = 128
            nc.vector.memset(st_dynX[96:97, :], 1.0)

            # --- LPC sample history (shifted each step via PE shift-matrix)
            st_ring = singles.tile([M, RC], DT32)    # SBUF copy (matmul rhs)
            nc.vector.memset(st_ring, 0.0)
            # --- persistent PSUM: double-buffered p
            ps_pp = psR.tile([33, 2 * RC], DT32)
            p_bufs = [ps_pp[32:33, 0:RC], ps_pp[32:33, RC:2 * RC]]
            nc.vector.memset(p_bufs[0], 128.0)
            prev_p = p_bufs[0]

            for t in range(t_steps):
                # ---- gumbel weight tile for this step
                w_t = wpool.tile([RC, 2 * RC], DT32, tag="wt")
                nc.sync.dma_start(out=w_t, in_=d_wT[t])

                # ---- GRU-A matmuls
                ps_rz = psA.tile([RC, 2 * HA], DT32, tag="rz")
                for c in range(6):
                    o = ps_rz[:, _ts(c)]
                    nc.tensor.matmul(o, sb_wxac[:, _ts(c)], sb_cond,
                                     start=True, stop=False)
                    for kc in range(3):
                        nc.tensor.matmul(o, sb_wha[:, kc, _ts(c)],
                                         st_A16[:, _ts(kc)], start=False, stop=False)
                    nc.tensor.matmul(o, sb_dw1[:, _ts(c)], st_dynX,
                                     start=False, stop=False)
                    nc.tensor.matmul(o, sb_dw2[:, _ts(c)], st_dynX,
                                     start=False, stop=True)
                ps_P1 = psB.tile([RC, HA], DT32, tag="p1")      # gx_n
                for c in range(3):
                    o = ps_P1[:, _ts(c)]
                    nc.tensor.matmul(o, sb_wxac[:, _ts(6 + c)], sb_cond,
                                     start=True, stop=False)
                    nc.tensor.matmul(o, sb_dw1[:, _ts(6 + c)], st_dynX,
                                     start=False, stop=False)
                    nc.tensor.matmul(o, sb_dw2[:, _ts(6 + c)], st_dynX,
                                     start=False, stop=True)
                ps_P2 = psC.tile([RC, HA], DT32, tag="p2")      # G = 0.5*gh_n
                for c in range(3):
                    o = ps_P2[:, _ts(c)]
                    for kc in range(3):
                        nc.tensor.matmul(o, sb_wha[:, kc, _ts(6 + c)],
                                         st_A16[:, _ts(kc)],
                                         start=(kc == 0), stop=(kc == 2))

                # ---- GRU-A gates
                t_rz = work.tile([RC, 2 * HA], DT16, tag="trz")
                nc.scalar.activation(t_rz, ps_rz, TANH, scale=0.5)
                one_tr = work.tile([RC, HA], DT16, tag="onetr")
                nc.vector.tensor_scalar(one_tr, t_rz[:, :HA], 1.0, None, op0=ALU.add)
                u2 = work.tile([RC, HA], DT32, tag="u2")
                nc.vector.tensor_mul(u2, one_tr, ps_P2)        # (1+t_r)*G
                n_arg = work.tile([RC, HA], DT32, tag="narg")
                nc.vector.tensor_add(n_arg, u2, ps_P1)
                n_t = work.tile([RC, HA], DT16, tag="nt")
                nc.scalar.activation(n_t, n_arg, TANH)
                # A' = (.5A + n) + t_z*(.5A - n)   (all fp16 sbuf -> 2x mode)
                d_t = work.tile([RC, HA], DT16, tag="dt")
                nc.vector.scalar_tensor_tensor(d_t, st_A16, 0.5, n_t,
                                               op0=ALU.mult, op1=ALU.subtract)
                g_t = work.tile([RC, HA], DT16, tag="gt")
                nc.vector.scalar_tensor_tensor(g_t, st_A16, 0.5, n_t,
                                               op0=ALU.mult, op1=ALU.add)
                f_t = work.tile([RC, HA], DT16, tag="ft")
                nc.vector.tensor_mul(f_t, t_rz[:, HA:], d_t)
                nc.vector.tensor_add(st_A16, f_t, g_t)          # state update

                # ---- GRU-B (input is A' via pre-halved Wxb)
                ps_b = psD.tile([HB, 4 * RC], DT32, tag="b")
                rb = ps_b[:, 0:RC]
                zb = ps_b[:, RC:2 * RC]
                P1b = ps_b[:, 2 * RC:3 * RC]
                P2b = ps_b[:, 3 * RC:4 * RC]
                # each accumulation group must fully close before the next
                # one opens in the same PSUM bank
                for (o, col, gh) in ((rb, slice(0, HB), True),
                                     (zb, slice(HB, 2 * HB), True),
                                     (P1b, slice(2 * HB, 3 * HB), False)):
                    for kc in range(3):
                        nc.tensor.matmul(o, sb_wxb[:, kc, col],
                                         st_A16[:, _ts(kc)], start=(kc == 0), stop=False)
                    nc.tensor.matmul(o, sb_bb2[:, col], sb_ones2,
                                     start=False, stop=not gh)
                    if gh:
                        nc.tensor.matmul(o, sb_whb[:, col], st_hbE[:HB, :],
                                         start=False, stop=True)
                nc.tensor.matmul(P2b, sb_whb[:, 2 * HB:], st_hbE[:HB, :],
                                 start=True, stop=True)

                t_rzb = work.tile([HB, 2 * RC], DT16, tag="trzb")
                nc.scalar.activation(t_rzb, ps_b[:, 0:2 * RC], TANH, scale=0.5)
                one_trb = work.tile([HB, RC], DT16, tag="onetrb")
                nc.vector.tensor_scalar(one_trb, t_rzb[:, 0:RC], 1.0, None, op0=ALU.add)
                u2b = work.tile([HB, RC], DT32, tag="u2b")
                nc.vector.tensor_mul(u2b, one_trb, P2b)
                n_argb = work.tile([HB, RC], DT32, tag="nargb")
                nc.vector.tensor_add(n_argb, u2b, P1b)
                n_tb = work.tile([HB, RC], DT32, tag="ntb")
                nc.scalar.activation(n_tb, n_argb, TANH)
                d_b = work.tile([HB, RC], DT32, tag="db")
                nc.vector.scalar_tensor_tensor(d_b, st_B32, 0.5, n_tb,
                                               op0=ALU.mult, op1=ALU.subtract)
                g_b = work.tile([HB, RC], DT32, tag="gb")
                nc.vector.scalar_tensor_tensor(g_b, st_B32, 0.5, n_tb,
                                               op0=ALU.mult, op1=ALU.add)
                f_b = work.tile([HB, RC], DT32, tag="fb")
                nc.vector.tensor_mul(f_b, t_rzb[:, RC:], d_b)
                nc.vector.tensor_add(st_B32, f_b, g_b)
                nc.vector.tensor_copy(st_hbE[:HB, :], st_B32)   # fp16 for matmul

                # ---- dual FC -> logits
                ps_lg = psE.tile([RC, 4 * RC], DT32, tag="lg")  # [l1c0 l1c1 l2c0 l2c1]
                for q in range(4):
                    nc.tensor.matmul(ps_lg[:, _ts(q)], sb_w12[:, _ts(q)], st_hbE,
                                     start=True, stop=True)
                t_l = work.tile([RC, 4 * RC], DT32, tag="tl")
                nc.scalar.activation(t_l, ps_lg, TANH)
                lgout = lgpool.tile([RC, 2 * RC], DT32, tag="lgout")
                for c in range(2):
                    q2 = work.tile([RC, RC], DT32, tag=f"q2{c}")
                    nc.vector.tensor_scalar(q2, t_l[:, _ts(2 + c)],
                                            sb_gg[:, 2 + c:3 + c], None, op0=ALU.mult)
                    nc.vector.scalar_tensor_tensor(lgout[:, _ts(c)], t_l[:, _ts(c)],
                                                   sb_gg[:, c:c + 1], q2,
                                                   op0=ALU.mult, op1=ALU.add)

                # ---- gumbel softmax expected index
                E_t = work.tile([RC, 2 * RC], DT32, tag="Et")
                nc.scalar.activation(E_t, lgout, EXP)
                Q_t = work.tile([RC, 2 * RC], DT32, tag="Qt")
                nc.vector.tensor_mul(Q_t, E_t, w_t)
                # psF layout: vb [:,0:128] | S0 [0:1,128:256] | S1 [0:1,256:384]
                #             | ring_ps [0:16, 384:512]
                ps_sm = psF.tile([RC, 4 * RC], DT32, tag="sm")
                S0 = ps_sm[0:1, RC:2 * RC]
                S1 = ps_sm[0:1, 2 * RC:3 * RC]
                nc.tensor.matmul(S0, sb_sidx[:, 0:1], Q_t[:, 0:RC],
                                 start=True, stop=False)
                nc.tensor.matmul(S0, sb_sidx[:, 2:3], Q_t[:, RC:],
                                 start=False, stop=True)
                nc.tensor.matmul(S1, sb_sidx[:, 1:2], Q_t[:, 0:RC],
                                 start=True, stop=False)
                nc.tensor.matmul(S1, sb_sidx[:, 3:4], Q_t[:, RC:],
                                 start=False, stop=True)
                rec = work.tile([1, RC], DT32, tag="rec")
                nc.vector.reciprocal(rec, S0)
                soft = work.tile([1, RC], DT32, tag="soft")
                nc.vector.tensor_mul(soft, S1, rec)
                e_i = work.tile([1, RC], DTI32, tag="ei")
                if ROUND_MODE == "rne":
                    nc.vector.tensor_copy(e_i, soft)
                else:
                    nc.vector.tensor_scalar(e_i, soft, 0.5, None, op0=ALU.add)
                e_f = work.tile([1, RC], DT32, tag="ef")
                nc.vector.tensor_copy(e_f, e_i)
                nc.vector.tensor_copy(st_dynX[64:65, :], e_f)
                # s = p + e
                s_sb = work.tile([1, RC], DT32, tag="ssb")
                nc.vector.scalar_tensor_tensor(s_sb, e_f, 1.0, prev_p,
                                               op0=ALU.bypass, op1=ALU.add)
                nc.vector.tensor_copy(st_dynX[32:33, :], s_sb)

                # ---- shift the LPC history: ring' = Shift.ring + unit15.s
                ring_ps = ps_sm[0:M, 3 * RC:4 * RC]
                nc.tensor.matmul(ring_ps, sb_shift[:, 0:M], st_ring,
                                 start=True, stop=False)
                nc.tensor.matmul(ring_ps, sb_shift[0:1, M:2 * M], s_sb,
                                 start=False, stop=True)

                # ---- p(t+1) via LPC dot + exact thresholds
                q16 = work.tile([M, RC], DT32, tag="q16")
                nc.vector.tensor_mul(q16, sb_lpcr, ring_ps)
                nc.vector.tensor_copy(st_ring, ring_ps)    # for next step's shift
                vb = ps_sm[:, 0:RC]
                nc.tensor.matmul(vb, sb_ones16, q16, start=True, stop=True)
                cmp0 = work.tile([RC, RC], DT16, tag="cmp0")
                nc.vector.tensor_scalar(cmp0, vb, sb_thrp[:, 0:1], None, op0=ALU.is_ge)
                cmp1 = work.tile([RC, RC], DT16, tag="cmp1")
                nc.vector.tensor_scalar(cmp1, vb, sb_thrp[:, 1:2], None, op0=ALU.is_ge)
                ps_p = p_bufs[(t + 1) % 2]
                nc.tensor.matmul(ps_p, sb_ones128, cmp0, start=True, stop=False)
                nc.tensor.matmul(ps_p, sb_ones128, cmp1, start=False, stop=True)
                nc.vector.tensor_copy(st_dynX[0:1, :], ps_p)
                prev_p = ps_p

                # ---- store logits
                nc.sync.dma_start(out=d_out[t], in_=lgout)
                if debug:
                    dbg = work.tile([1, 4 * RC], DT32, tag="dbg")
                    nc.vector.tensor_copy(dbg[:, 0:RC], soft)
                    nc.vector.tensor_copy(dbg[:, RC:2 * RC], e_f)
                    nc.vector.tensor_copy(dbg[:, 2 * RC:3 * RC], ps_p)
                    nc.vector.tensor_copy(dbg[:, 3 * RC:4 * RC], s_sb)
                    nc.sync.dma_start(out=d_dbg[t].rearrange("a b -> (a b)")[None, :], in_=dbg)

    nc.finalize()
    return nc


# ------------------------------------------------------------------ host prep

def _prep_shared_weights(Wxa, Wha, ba, Wxb, Whb, bb, W1, b1, g1, W2, b2, g2):
    """Build the replicated (same on every core) device weight arrays."""
    Wxa = Wxa.astype(f32); Wha = Wha.astype(f32)
    Wxb = Wxb.astype(f32); Whb = Whb.astype(f32)

    # Wha: rhs is A = 2*ha -> scale rz cols by 0.5; n cols by 0.25 (G = .5*gh_n)
    wha_dev = Wha.copy()
    wha_dev[:, :2 * HA] *= 0.5
    wha_dev[:, 2 * HA:] *= 0.25
    whaT = wha_dev.astype(f16).reshape(3, RC, 3 * HA).transpose(1, 0, 2).reshape(RC, 9 * HA)

    wxacT = Wxa[:COND].astype(f16)                      # [128, 1152]
    dhi, dlo = _f16x2(Wxa[COND:])                       # [3, 1152] each
    bhi, blo = _f16x2(ba[None, :].astype(f32))          # [1, 1152]
    dynW1 = np.zeros((97, 3 * HA), f16)
    dynW2 = np.zeros((97, 3 * HA), f16)
    dynW1[0], dynW1[32], dynW1[64], dynW1[96] = dhi[0], dhi[1], dhi[2], bhi[0]
    dynW2[0], dynW2[32], dynW2[64], dynW2[96] = dlo[0], dlo[1], dlo[2], blo[0]

    # Wxb: rhs is A = 2*ha2 (GRU-B input is ha2) -> scale all cols by 0.5.
    wxb_dev = 0.5 * Wxb
    wxbT = wxb_dev.astype(f16).reshape(3, RC, 3 * HB).transpose(1, 0, 2).reshape(RC, 9 * HB)
    bbhi, bblo = _f16x2(bb[None, :].astype(f32))
    bb2 = np.concatenate([bbhi, bblo], axis=0)          # [2, 48]

    whb_dev = Whb.copy()
    whb_dev[:, :2 * HB] *= 0.5
    whb_dev[:, 2 * HB:] *= 0.25
    whbT = whb_dev.astype(f16)                          # [16, 48]

    # W1/W2: rhs is B = 2*hb2 -> 0.5; bias rows via ones rows of hbE
    w12 = np.concatenate([0.5 * W1.astype(f32), 0.5 * W2.astype(f32)], axis=1)  # [16, 512]
    b12 = np.concatenate([b1.astype(f32), b2.astype(f32)])[None, :]             # [1, 512]
    b12hi, b12lo = _f16x2(b12)
    w12T = np.concatenate([w12.astype(f16), b12hi, b12lo], axis=0)              # [18, 512]

    thr = _compute_thresholds()
    thrp = np.stack([thr[:RC],
                     np.concatenate([thr[RC:], [np.float32(3e38)]])], axis=1).astype(f32)

    sidx = np.zeros((RC, 4), f32)
    sidx[:, 0] = 1.0
    sidx[:, 1] = np.arange(RC, dtype=f32)
    sidx[:, 2] = 1.0
    sidx[:, 3] = np.arange(RC, dtype=f32) + RC

    ggcol = np.zeros((RC, 4), f32)
    ggcol[:, 0] = g1[:RC]; ggcol[:, 1] = g1[RC:]
    ggcol[:, 2] = g2[:RC]; ggcol[:, 3] = g2[RC:]

    # ring' = ShiftM.T @ ring + unit15.T @ s:  ring'[m] = ring[m+1], ring'[15] = s
    shiftm = np.zeros((M, 2 * M), f32)
    for m_ in range(M - 1):
        shiftm[m_ + 1, m_] = 1.0
    shiftm[0, M + M - 1] = 1.0

    return dict(whaT=whaT, wxacT=wxacT, dynW1=dynW1, dynW2=dynW2, wxbT=wxbT,
                bb2W=bb2.astype(f16), whbT=whbT, w12T=w12T, thrp=thrp,
                sidx=sidx, ggcol=ggcol, shiftm=shiftm)


def _prep_core_inputs(c, cond, lpc, w_all, shared, t_steps):
    rs = slice(c * RC, (c + 1) * RC)
    condT = np.ascontiguousarray(cond[rs].T).astype(f16)          # [128, 128]

    lpcfm = np.ascontiguousarray(lpc[rs].T).astype(f32)           # [16, 128]

    wT = w_all[:, rs, :]                                          # [T, 128, 256]
    wT = np.ascontiguousarray(
        wT.reshape(t_steps, RC, 2, RC).transpose(0, 3, 2, 1)      # [t, p, c, r]
    ).reshape(t_steps, RC, 2 * RC)

    m = dict(shared)
    m.update(cond16=condT, lpcfm=lpcfm, wT=wT.astype(f32))
    return m


def _ensure_ntff_hook():
    """bass_utils' trace path needs antenv.axon_hooks, which this image's
    antenv lacks. Synthesize it and register the ctypes NTFF hook that
    trn_agent_boot already knows how to build (env-native profiling path)."""
    try:
        from antenv.axon_hooks import get_axon_ntff_profile_hook  # noqa: F401
        return
    except ImportError:
        pass
    import sys
    import types
    import antenv
    mod = types.ModuleType("antenv.axon_hooks")
    holder = [None]
    mod.set_axon_ntff_profile_hook = lambda h: holder.__setitem__(0, h)
    mod.get_axon_ntff_profile_hook = lambda: holder[0]
    sys.modules["antenv.axon_hooks"] = mod
    antenv.axon_hooks = mod
    try:
        from trn_agent_boot.trn_boot import _ntff_profile_via_ctypes
        so = "/opt/axon/libaxon_pjrt.so"
        if os.path.exists(so):
            hook = _ntff_profile_via_ctypes(so)
            if hook is not None:
                mod.set_axon_ntff_profile_hook(hook)
    except Exception:
        pass


_NC_CACHE = {}


def kernel(frames_features, lpc_coeffs, gumbel_u, Wf1, bf1, Wf2, bf2,
           Wxa, Wha, ba, Wxb, Whb, bb, W1, b1, g1, W2, b2, g2):
    global last_exec_time_ns
    t_steps = T_STEPS

    feat = np.asarray(frames_features, f32).reshape(R, NF)
    lpc = np.asarray(lpc_coeffs, f32).reshape(R, M)
    u_all = np.asarray(gumbel_u, f32)[:t_steps]

    # frame-rate conditioning network (tiny, fp32, matches reference numerics)
    cond = np.tanh(
        (np.tanh((feat @ np.asarray(Wf1, f32) + np.asarray(bf1, f32)).astype(f32)).astype(f32)
         @ np.asarray(Wf2, f32) + np.asarray(bf2, f32)).astype(f32)).astype(f32)

    # gumbel weights w' = 1/log(u)
    w_all = (f32(1.0) / np.log(u_all).astype(f32)).astype(f32)    # [T, R, K]

    shared = _prep_shared_weights(np.asarray(Wxa, f32), np.asarray(Wha, f32),
                                  np.asarray(ba, f32), np.asarray(Wxb, f32),
                                  np.asarray(Whb, f32), np.asarray(bb, f32),
                                  np.asarray(W1, f32), np.asarray(b1, f32),
                                  np.asarray(g1, f32), np.asarray(W2, f32),
                                  np.asarray(b2, f32), np.asarray(g2, f32))

    in_maps = [_prep_core_inputs(c, cond, lpc, w_all, shared, t_steps)
               for c in range(N_CORES)]

    if t_steps not in _NC_CACHE:
        _NC_CACHE[t_steps] = _build_nc(t_steps)
    nc = _NC_CACHE[t_steps]

    trace = bool(int(os.environ.get("LPC_TRACE", "0")))
    if trace:
        _ensure_ntff_hook()
        try:
            res = run_bass_kernel_spmd(nc, in_maps, core_ids=list(range(N_CORES)),
                                       trace=True)
        except Exception as ex:  # profiling plumbing unavailable -> plain run
            print(f"trace run failed ({ex!r}); rerunning untraced")
            res = run_bass_kernel_spmd(nc, in_maps, core_ids=list(range(N_CORES)))
    else:
        res = run_bass_kernel_spmd(nc, in_maps, core_ids=list(range(N_CORES)))
    last_exec_time_ns = res.exec_time_ns

    # gather: out_lg[t, p, c*128+r] -> logits[r_global, t, k=c*128+p]
    out = np.empty((R, t_steps, K), f32)
    for c in range(N_CORES):
        o = np.asarray(res.results[c]["out_lg"])                 # [T, 128, 256]
        o = o.reshape(t_steps, RC, 2, RC).transpose(0, 2, 1, 3)  # [t, c2, p, r]
        o = o.reshape(t_steps, K, RC)                            # [t, k, r]
        out[c * RC:(c + 1) * RC] = o.transpose(2, 0, 1)          # [r, t, k]

    if t_steps == T:
        return np.ascontiguousarray(out.reshape(B, F, T, K).reshape(B, F * T, K))
    return out   # partial-T debug mode
